# revision 15
# baseline (speedup 1.0000x reference)
"""Trainium2 Bass kernel for the blocked-DCT corner-mask layer.

Math: for each 8x8 block B of the image, the reference computes
    coeffs = D^T B D        (2D DCT-II)
    out_c  = D (coeffs * mask_c) D^T   for 4 corner masks c
Each mask is an outer product of half-indicators, so with
    L = D[:, :4] @ D[:, :4].T   (symmetric projection),  H = I - L
the whole pipeline collapses to
    out_0 = L B L,  out_1 = L B H,  out_2 = H B L,  out_3 = H B H.

Per-8-row/8-col application over a full 512x512 image is multiplication by
the 128x128 block-diagonal BDL = blockdiag(L x 16) (symmetric) on either
side.  On-chip per [128, 512] tile X:
    XT_c   = transpose(X[:, c*128:(c+1)*128])            (PE transpose)
    [C|CH] = XT_c^T @ [BDL | BDH]                        (PE, N=256, f32r)
             = [X@BDL | X@BDH]  (column transform + complement)
    O0 = BDL @ C, O1 = BDL @ CH, O2 = BDH @ C, O3 = BDH @ CH  (PE, N=512)

Sharding: data-parallel over batch, 4 batches (12 images) per core.
"""

import numpy as np

FULL_B, DCH, H, W = 32, 3, 512, 512
N_CORES = 8
B_PER_CORE = FULL_B // N_CORES       # 4
IMGS = B_PER_CORE * DCH              # 12 images per core
P = 128

_BUILT = {}


def _consts() -> np.ndarray:
    """[128, 384] = [I128 | BDL | BDH] constants, computed in float64."""
    N = 8
    x = np.arange(N, dtype=np.float64)[:, None]
    u = np.arange(N, dtype=np.float64)[None, :]
    alpha = np.full(N, np.sqrt(2.0 / N))
    alpha[0] = np.sqrt(1.0 / N)
    D = alpha[None, :] * np.cos(np.pi * u * (2.0 * x + 1.0) / (2.0 * N))
    L = D[:, :4] @ D[:, :4].T
    Hm = np.eye(N) - L
    BDL = np.kron(np.eye(16), L).astype(np.float32)
    BDH = np.kron(np.eye(16), Hm).astype(np.float32)
    ident = np.eye(P, dtype=np.float32)
    return np.ascontiguousarray(np.concatenate([ident, BDL, BDH], axis=1))


def _body(ctx, tc, o_ap, x_ap, c_ap, n_imgs, use_f32r=True):
    import concourse.mybir as mybir

    nc = tc.nc
    f32 = mybir.dt.float32
    f32r = mybir.dt.float32r
    mmdt = f32r if use_f32r else f32

    cpool = ctx.enter_context(tc.tile_pool(name="const", bufs=1))
    cst = cpool.tile([P, 384], f32)
    nc.sync.dma_start(cst[:], c_ap[:, :])
    # fp32r-typed copy of the constants: compute engines must produce
    # (round) fp32r data before a fp32r matmul may consume it.
    cst_r = cpool.tile([P, 384], mmdt, name="cst_r")
    nc.vector.tensor_copy(cst_r[:], cst[:])
    ident = cst_r[:, 0:128]
    BDL = cst_r[:, 128:256]
    BDH = cst_r[:, 256:384]
    BDLH = cst_r[:, 128:384]  # packed [BDL | BDH] rhs, N=256

    sb = ctx.enter_context(tc.tile_pool(name="sb", bufs=1))
    ps = ctx.enter_context(tc.tile_pool(name="ps", bufs=1, space="PSUM"))

    def front(i):
        """input DMA + transposes + cast + C/CH matmuls + cch copy."""
        img, t = divmod(i, 4)
        row = img * 512 + t * 128
        x_sb = sb.tile([P, 512], mmdt, tag="x", bufs=6, name=f"x_{i}")
        nc.gpsimd.dma_start(x_sb[:], x_ap[row : row + 128, :])  # SWDGE ring

        # transpose the 4 chunks of X into one PSUM bank
        xt_ps = ps.tile([P, 512], mmdt, tag="xt", bufs=2, name=f"xtp_{i}")
        for c in range(4):
            nc.tensor.transpose(
                xt_ps[:, 128 * c : 128 * (c + 1)],
                x_sb[:, 128 * c : 128 * (c + 1)],
                ident,
            )
        xt_sb = sb.tile([P, 512], mmdt, tag="xts", bufs=6, name=f"xts_{i}")
        nc.vector.tensor_copy(xt_sb[:], xt_ps[:])  # DVE (rounds to fp32r)

        # C/CH packed matmuls: chunk c -> cols [256c, 256c+256)
        # layout: [C0|CH0|C1|CH1|C2|CH2|C3|CH3], each 128 wide
        cch_ps = ps.tile([P, 1024], f32, tag="cch", bufs=1, name=f"cchp_{i}")
        for c in range(4):
            nc.tensor.matmul(
                cch_ps[:, 256 * c : 256 * (c + 1)],
                lhsT=xt_sb[:, 128 * c : 128 * (c + 1)],
                rhs=BDLH,
                start=True,
                stop=True,
            )
        # single contiguous copy; output matmuls read strided C/CH views
        cch_sb = sb.tile([P, 1024], mmdt, tag="cchs", bufs=4, name=f"cch_{i}")
        nc.scalar.copy(cch_sb[:], cch_ps[:])  # ACT
        return cch_sb

    def back(i, cch_sb):
        """output matmuls + split copies + output DMAs."""
        img, t = divmod(i, 4)
        cch_v = cch_sb[:].rearrange("p (c s l) -> p c s l", c=4, s=2, l=128)
        c_v = cch_v[:, :, 0, :]
        ch_v = cch_v[:, :, 1, :]

        oa_ps = ps.tile([P, 1024], f32, tag="oa", bufs=1, name=f"oap_{i}")
        nc.tensor.matmul(
            oa_ps[:, 0:512], lhsT=BDL, rhs=c_v, start=True, stop=True
        )  # O0 = BDL @ C
        nc.tensor.matmul(
            oa_ps[:, 512:1024], lhsT=BDL, rhs=ch_v, start=True, stop=True
        )  # O1 = BDL @ CH
        ob_ps = ps.tile([P, 1024], f32, tag="ob", bufs=1, name=f"obp_{i}")
        nc.tensor.matmul(
            ob_ps[:, 0:512], lhsT=BDH, rhs=c_v, start=True, stop=True
        )  # O2 = BDH @ C
        nc.tensor.matmul(
            ob_ps[:, 512:1024], lhsT=BDH, rhs=ch_v, start=True, stop=True
        )  # O3 = BDH @ CH

        # split copies: DVE and ACT each drain one bank of the pair,
        # concurrently (different banks), so PSUM frees ~2x sooner
        oa_sb = sb.tile([P, 1024], f32, tag="oas", bufs=6, name=f"oas_{i}")
        nc.vector.tensor_copy(oa_sb[:, 0:512], oa_ps[:, 0:512])  # DVE
        nc.scalar.copy(oa_sb[:, 512:1024], oa_ps[:, 512:1024])  # ACT
        ob_sb = sb.tile([P, 1024], f32, tag="obs", bufs=6, name=f"obs_{i}")
        nc.vector.tensor_copy(ob_sb[:, 0:512], ob_ps[:, 0:512])  # DVE
        nc.scalar.copy(ob_sb[:, 512:1024], ob_ps[:, 512:1024])  # ACT

        # split output DMAs across the two HWDGE rings (SP + ACT)
        for ci, (osb, col, eng) in enumerate(
            [
                (oa_sb, 0, nc.sync),
                (oa_sb, 512, nc.sync),
                (ob_sb, 0, nc.scalar),
                (ob_sb, 512, nc.scalar),
            ]
        ):
            orow = (ci * n_imgs + img) * 512 + t * 128
            eng.dma_start(o_ap[orow : orow + 128, :], osb[:, col : col + 512])

    # one-stage software skew: tile i's output stage is emitted after
    # tile i+1's front stage, keeping PE fed while PSUM banks drain
    ntiles = n_imgs * 4
    pending = None
    for i in range(ntiles):
        cch = front(i)
        if pending is not None:
            back(i - 1, pending)
        pending = cch
    back(ntiles - 1, pending)


def _build(n_imgs=IMGS, use_f32r=True):
    key = (n_imgs, use_f32r)
    if key in _BUILT:
        return _BUILT[key]
    from contextlib import ExitStack

    import concourse.bacc as bacc
    import concourse.mybir as mybir
    import concourse.tile as tile

    f32 = mybir.dt.float32
    xdt = mybir.dt.float32r if use_f32r else f32
    nc = bacc.Bacc(
        "TRN2", target_bir_lowering=False, debug=False, num_devices=N_CORES
    )
    x_d = nc.dram_tensor("x", (n_imgs * 512, 512), xdt, kind="ExternalInput")
    c_d = nc.dram_tensor("cst", (P, 384), f32, kind="ExternalInput")
    o_d = nc.dram_tensor("out", (4 * n_imgs * 512, 512), f32, kind="ExternalOutput")

    with tile.TileContext(nc) as tc:
        with ExitStack() as ctx:
            _body(ctx, tc, o_d.ap(), x_d.ap(), c_d.ap(), n_imgs, use_f32r)
    nc.compile()
    _BUILT[key] = nc
    return nc


def _run(x, trace=False, use_f32r=True):
    """x: (32, 3, 512, 512) float32. Returns (out, exec_time_ns)."""
    from concourse import bass_utils

    nc = _build(IMGS, use_f32r)
    consts = _consts()
    in_maps = []
    for k in range(N_CORES):
        xs = x[k * B_PER_CORE : (k + 1) * B_PER_CORE].reshape(IMGS * 512, 512)
        in_maps.append({"x": np.ascontiguousarray(xs), "cst": consts})
    res = bass_utils.run_bass_kernel_spmd(
        nc, in_maps, core_ids=list(range(N_CORES)), trace=trace
    )
    outs = []
    for k in range(N_CORES):
        o = res.results[k]["out"].reshape(4, B_PER_CORE, DCH, H, W)
        outs.append(o)
    full = np.concatenate(outs, axis=1)  # (4, 32, 3, 512, 512)
    return full, res.exec_time_ns


def kernel(**inputs) -> np.ndarray:
    x = np.ascontiguousarray(np.asarray(inputs["x"], dtype=np.float32))
    assert x.shape == (FULL_B, DCH, H, W), x.shape
    out, _ = _run(x, trace=False)
    return out
